# revision 43
# baseline (speedup 1.0000x reference)
"""nn_AttentionOut_63711544869147 — causal multi-head attention + output projection,
distributed over 8 Trainium2 NeuronCores.

Module: out = softmax(causal(Q K^T / sqrt(d))) V @ W_O + b_O, returned with the
(unchanged) residual: reference returns the tuple (residual, out).

Sharding (8 cores = 2 batches x 4 head-groups of 4 heads, SPMD single program):
  each core computes full causal attention for its batch over its 4 heads and
  a partial projection  sum_{h in group} z_h @ W_O[h]  ->  pout [2048, 1024].
  The host sums the 4 head-group partials per batch (the "all-reduce" of the
  row-sharded W_O product), adds b_O, and passes the residual through.

The kernel is emitted as ONE continuous software pipeline over all 80
(strip, head-pair, kv-tile) tiles, so no engine queue ever head-of-line
blocks at a pair or strip boundary:
  per tile: scores_T[kv,q] = K^T_tile.T @ Q^T_strip for both heads of the
  pair concurrently (64-deep matmuls in disjoint PE row groups, shared
  [128, 2, 512] fp32 score tile), then exp(scores/8) on EITHER the ACT
  engine (exact) or the DVE (Schraudolph bf16 bit-trick via one
  tensor_scalar into int16) — the split balances the two queues; diagonal
  tiles get one GpSimd mask-multiply over both heads' straddle blocks.
  PV accumulation (z_ext[65,q] += V_ext.T @ expP, ones column = softmax
  denominator) is emitted with an asymmetric lag: L_HI tiles behind the
  QK/exp stream mid-pair (so a PV stalled on a z-bank recycle never blocks
  the next QKs in the PE FIFO), catching up to lag L_LO by the pair's last
  tile (so the normalize chain starts early at the boundary).
  normalize: 1/denom via DVE approx-reciprocal, broadcast across the 64
  d-partitions by a GpSimd partition_broadcast (no DRAM bounce), then the
  zn = z * recip multiplies two pipeline steps later.
  projection (zn_T @ W_O, 256-deep) is event-scheduled after its strip's
  normalizes; pout is stored fp16 via PSUM-evacuation copies split between
  the Scalar and Vector queues, with the row DMAs on the Sync queue.
"""

import numpy as np

import concourse.bass as bass
import concourse.bacc as bacc
import concourse.tile as tile
from concourse import mybir
from concourse.bass_utils import run_bass_kernel_spmd

F32 = mybir.dt.float32
F16 = mybir.dt.float16
I16 = mybir.dt.int16
BF16 = mybir.dt.bfloat16

N_CORES = 8
N_HEADS = 16
H = 4          # heads per core
S = 2048
D = 64
P = 128
D_MODEL = 1024
NSTRIP = 4     # q strips of 512
QW = 512       # strip width
L_HI = 7       # PV lag mid-pair (tiles)
L_LO = 2       # PV lag at pair end

# bf16-bit-trick exp: bits16 = rne(score * (0.125*2^7/ln2) + (127*2^7 - 7.25))
SCHR_A = float(0.125 * 128.0 / np.log(2.0))
SCHR_B = float(127 * 128 - 7.25)


def dve_exp(s, j, t):
    """Which tiles' exp runs on DVE (Schraudolph) instead of ACT (exact)."""
    return t % 3 == 1 and not (s == 3 and t in (10, 13))


def build_program():
    MMDT = BF16
    nc = bacc.Bacc(target_bir_lowering=False)

    qT = nc.dram_tensor("qT", [H, D, S], MMDT, kind="ExternalInput")
    kT = nc.dram_tensor("kT", [H, D, S], MMDT, kind="ExternalInput")
    vx = nc.dram_tensor("vx", [H, P, 16, D + 1], MMDT, kind="ExternalInput")
    wo = nc.dram_tensor("wo", [2 * P, D_MODEL], MMDT, kind="ExternalInput")
    tri2 = nc.dram_tensor("tri2", [P, 2 * P], MMDT, kind="ExternalInput")
    pout = nc.dram_tensor("pout", [S, D_MODEL], F16, kind="ExternalOutput")

    with tile.TileContext(nc) as tc:
        with (
            tc.tile_pool(name="persist", bufs=1) as persist,
            tc.tile_pool(name="expp", bufs=16) as expp,
            tc.tile_pool(name="rcpp", bufs=4) as rcpp,
            tc.tile_pool(name="rbp", bufs=4) as rbp,
            tc.tile_pool(name="outp", bufs=6) as outp,
            tc.tile_pool(name="znp", bufs=6) as znp,
            tc.tile_pool(name="scps", bufs=3, space="PSUM") as scps,
            tc.tile_pool(name="zps", bufs=2, space="PSUM") as zps,
        ):
            # ---- persistent loads (everything stays SBUF-resident) ----
            # spread across the three DMA-capable queues; ordered so strip 0 /
            # head-pair 0 can start as early as possible
            qT_sb = [None, None]
            kT_sb = [None, None]
            wo_sb = [None, None]
            vext_sb = [None] * H

            # strip 0 only needs the first 512 kv/q columns: load those chunks
            # first so the pipeline starts ~2us earlier, then the remainders
            for j in range(2):
                kT_sb[j] = persist.tile([P, S], MMDT, tag=f"kT{j}", name=f"kT{j}")
                qT_sb[j] = persist.tile([P, S], MMDT, tag=f"qT{j}", name=f"qT{j}")
            tri_sb = persist.tile([P, 2, P], MMDT, tag="tri", name="tri_sb")
            for h in range(H):
                vext_sb[h] = persist.tile([P, 16, D + 1], MMDT, tag=f"vext{h}", name=f"vext{h}")

            nc.sync.dma_start(
                kT_sb[0][:, 0:P],
                kT[0:2, :, 0:P].rearrange("h d s -> (h d) s"))
            nc.sync.dma_start(
                kT_sb[0][:, P:QW],
                kT[0:2, :, P:QW].rearrange("h d s -> (h d) s"))
            nc.scalar.dma_start(
                qT_sb[0][:, 0:QW],
                qT[0:2, :, 0:QW].rearrange("h d s -> (h d) s"))
            nc.gpsimd.dma_start(tri_sb[:], tri2[:].rearrange("p (a b) -> p a b", a=2))
            for h in (0, 1):
                nc.gpsimd.dma_start(vext_sb[h][:], vx[h])
            nc.sync.dma_start(
                kT_sb[1][:, 0:QW],
                kT[2:4, :, 0:QW].rearrange("h d s -> (h d) s"))
            nc.scalar.dma_start(
                qT_sb[1][:, 0:QW],
                qT[2:4, :, 0:QW].rearrange("h d s -> (h d) s"))
            for h in (2, 3):
                nc.gpsimd.dma_start(vext_sb[h][:], vx[h])
            nc.sync.dma_start(
                kT_sb[0][:, QW:S],
                kT[0:2, :, QW:S].rearrange("h d s -> (h d) s"))
            nc.scalar.dma_start(
                qT_sb[0][:, QW:S],
                qT[0:2, :, QW:S].rearrange("h d s -> (h d) s"))
            nc.sync.dma_start(
                kT_sb[1][:, QW:S],
                kT[2:4, :, QW:S].rearrange("h d s -> (h d) s"))
            nc.scalar.dma_start(
                qT_sb[1][:, QW:S],
                qT[2:4, :, QW:S].rearrange("h d s -> (h d) s"))
            for j in range(2):
                wo_sb[j] = persist.tile([P, D_MODEL], MMDT, tag=f"wo{j}", name=f"wo{j}")
                nc.sync.dma_start(wo_sb[j][:], wo[P * j : P * (j + 1), :])

            # ---- global pipeline schedule ----
            tiles = []        # (pair_idx, t)
            pair_info = []    # per pair: s, j, nt, start step
            for s in range(NSTRIP):
                for j in range(2):
                    nt = 4 * s + 4
                    pair_info.append(
                        {"s": s, "j": j, "nt": nt, "start": len(tiles),
                         "z": None, "ex": {}, "zn": None}
                    )
                    for t in range(nt):
                        tiles.append((len(pair_info) - 1, t))
            n_steps = len(tiles)
            zn_strip = {}     # strip -> [zn_j0, zn_j1]
            actions = {}      # step -> list of closures

            def at(step, fn):
                actions.setdefault(step, []).append(fn)

            def emit_qk_exp(p, t):
                info = pair_info[p]
                s, j, nt = info["s"], info["j"], info["nt"]
                q0 = s * QW
                li = max(0, (t - 4 * s)) * P
                if t == 0:
                    info["z"] = [
                        zps.tile([D + 1, QW], F32, tag="z", name=f"z{x}")
                        for x in ("A", "B")
                    ]
                if info["zn"] is None:
                    zn = [
                        znp.tile([P, QW], MMDT, tag=f"zn{jj}", name=f"zn{jj}")
                        for jj in range(2)
                    ] if j == 0 else zn_strip[s]
                    zn_strip[s] = zn
                    info["zn"] = zn
                sc = scps.tile([P, 2, QW], F32, tag="sc", name="sc")
                # the two heads' 64-deep QK matmuls target disjoint PE row
                # groups (rows 0-63 / 64-127) and run concurrently
                for side in (0, 1):
                    off = side * D
                    nc.tensor.matmul(
                        sc[:, side, li:QW],
                        kT_sb[j][off : off + D, t * P : (t + 1) * P],
                        qT_sb[j][off : off + D, q0 + li : q0 + QW],
                        start=True,
                        stop=True,
                    )
                ex = expp.tile([P, 2, QW], MMDT, tag="ex", name="ex")
                if dve_exp(s, j, t):
                    nc.vector.tensor_scalar(
                        ex[:, :, li:QW].bitcast(I16), sc[:, :, li:QW],
                        SCHR_A, SCHR_B,
                        mybir.AluOpType.mult, mybir.AluOpType.add,
                    )
                else:
                    nc.scalar.activation(
                        ex[:, :, li:QW], sc[:, :, li:QW],
                        mybir.ActivationFunctionType.Exp, scale=0.125,
                    )
                info["ex"][t] = ex

            def emit_mask(p, t):
                """one mask multiply over both heads' straddle blocks;
                scheduled two steps after the exp so the Vector queue never
                blocks waiting for an ACT-engine exp"""
                info = pair_info[p]
                li = (t - 4 * info["s"]) * P
                m = info["ex"][t][:, :, li : li + P]
                nc.vector.tensor_mul(m, m, tri_sb[:])

            def emit_pv(p, t):
                info = pair_info[p]
                s, nt = info["s"], info["nt"]
                li = max(0, (t - 4 * s)) * P
                ex = info["ex"].pop(t)
                for side, h in ((0, 2 * info["j"]), (1, 2 * info["j"] + 1)):
                    nc.tensor.matmul(
                        info["z"][side][:, li:QW],
                        vext_sb[h][:, t, :],
                        ex[:, side, li:QW],
                        start=(t == 0),
                        stop=(t == nt - 1),
                    )

            def emit_norm1(p):
                """denominator -> reciprocal -> partition broadcast"""
                info = pair_info[p]
                info["rb"] = []
                for side in (0, 1):
                    dcp = rcpp.tile([1, QW], F32, tag="dcp", name="dcp")
                    nc.vector.tensor_copy(dcp[:], info["z"][side][D : D + 1, :])
                    rcp = rcpp.tile([1, QW], F32, tag="rcp", name="rcp")
                    # (custom-DVE op requires an SBUF input; PSUM reads garbage)
                    nc.vector.reciprocal_approx_fast(rcp[:], dcp[:])
                    rb_sb = rbp.tile([D, QW], F32, tag="rb_sb", name="rb_sb")
                    nc.gpsimd.partition_broadcast(rb_sb[:], rcp[:])
                    info["rb"].append(rb_sb)

            def emit_norm2(p):
                """zn = z * (1/denom); frees the pair's z banks"""
                info = pair_info[p]
                j = info["j"]
                for side in (0, 1):
                    off = side * D
                    nc.vector.tensor_mul(
                        info["zn"][j][off : off + D, :],
                        info["z"][side][0:D, :],
                        info["rb"][side][:],
                    )

            def emit_proj_mms(s, qb, store):
                zn_sb = zn_strip[s]
                # projection shares the score pool's 3-deep ring (the wops
                # pool's banks were donated to deepen QK/exp pipelining)
                ops = scps.tile([P, 2, 512], F32, tag="sc", name="wo_ps")
                for j2 in range(2):
                    for mt in range(2):
                        nc.tensor.matmul(
                            ops[:, mt, :],
                            zn_sb[j2][:, qb * P : (qb + 1) * P],
                            wo_sb[j2][:, mt * 512 : (mt + 1) * 512],
                            start=(j2 == 0),
                            stop=(j2 == 1),
                        )
                store.append(ops)

            def emit_proj_copies(s, qb, store):
                ops = store.pop(0)
                ot = outp.tile([P, 2, 512], F16, tag="ot", name="ot")
                # one merged [128, 1024] evacuation copy per q block,
                # alternating between the Scalar and Vector queues; deferred
                # two steps behind the matmuls so it never waits in-queue
                eng = nc.scalar.copy if qb % 2 == 0 else nc.vector.tensor_copy
                eng(ot[:], ops[:])
                nc.sync.dma_start(
                    pout[(4 * s + qb) * P : (4 * s + qb + 1) * P, :],
                    ot[:].rearrange("p a b -> p (a b)"),
                )

            # schedule PVs with asymmetric lag + boundary events
            for g, (p, t) in enumerate(tiles):
                info = pair_info[p]
                nt, start = info["nt"], info["start"]
                if t >= 4 * info["s"]:
                    at(g + 2, (lambda p=p, t=t: emit_mask(p, t)))
                f = max(g + 2, min(g + L_HI, start + nt - 1 + (3 if nt <= 8 else L_LO)))
                at(f, (lambda p=p, t=t: emit_pv(p, t)))
                if t == nt - 1:
                    at(f, (lambda p=p: emit_norm1(p)))
                    at(f + 3, (lambda p=p: emit_norm2(p)))
                    if info["j"] == 1:
                        store = []
                        for qb in range(4):
                            # 2-step spacing: with wops bufs=2, block qb+1's
                            # matmuls reuse qb's banks, so qb's copies must be
                            # emitted first
                            at(f + 4 + 2 * qb,
                               (lambda s=info["s"], qb=qb, st=store:
                                emit_proj_mms(s, qb, st)))
                            at(f + 6 + 2 * qb,
                               (lambda s=info["s"], qb=qb, st=store:
                                emit_proj_copies(s, qb, st)))

            # ---- run the pipeline ----
            for g in range(n_steps):
                p, t = tiles[g]
                emit_qk_exp(p, t)
                for fn in actions.pop(g, ()):
                    fn()
            for g in sorted(actions):
                for fn in actions.pop(g):
                    fn()

    nc.finalize()
    return nc


_PROGRAM = None
LAST_RESULTS = None


def _get_program():
    global _PROGRAM
    if _PROGRAM is None:
        _PROGRAM = build_program()
    return _PROGRAM


def make_in_maps(q, k, v, W_O, n_cores=N_CORES):
    """Shard full inputs into per-core maps (core = batch*4 + head_group)."""
    import ml_dtypes
    mmdt = ml_dtypes.bfloat16
    q = np.ascontiguousarray(np.asarray(q, dtype=np.float32))
    k = np.ascontiguousarray(np.asarray(k, dtype=np.float32))
    v = np.ascontiguousarray(np.asarray(v, dtype=np.float32))
    W_O = np.ascontiguousarray(np.asarray(W_O, dtype=np.float32))
    B = q.shape[0]
    qT = np.ascontiguousarray(q.reshape(B, S, N_HEADS, D).transpose(0, 2, 3, 1))
    kT = np.ascontiguousarray(k.reshape(B, S, N_HEADS, D).transpose(0, 2, 3, 1))
    # v extended with a ones column (softmax denominator row) and pre-arranged
    # to the on-chip [partition, kv_tile, d+1] layout so the DMA is contiguous
    vh = v.reshape(B, S, N_HEADS, D).transpose(0, 2, 1, 3)  # [B, H, S, D]
    vext = np.concatenate(
        [vh, np.ones((B, N_HEADS, S, 1), dtype=np.float32)], axis=3
    ).reshape(B, N_HEADS, 16, P, D + 1).transpose(0, 1, 3, 2, 4)  # [B, Hh, P, 16, D+1]
    # mask[kv, q] = 1 iff kv <= q  (scores live transposed: partition=kv, free=q)
    tri = np.triu(np.ones((P, P), dtype=np.float32))
    tri2 = np.ascontiguousarray(np.concatenate([tri, tri], axis=1))
    in_maps = []
    for core in range(n_cores):
        b, g = core // 4, core % 4
        hs = slice(H * g, H * (g + 1))
        in_maps.append(
            {
                "qT": np.ascontiguousarray(qT[b, hs]).astype(mmdt),
                "kT": np.ascontiguousarray(kT[b, hs]).astype(mmdt),
                "vx": np.ascontiguousarray(vext[b, hs]).astype(mmdt),
                "wo": np.ascontiguousarray(W_O[hs].reshape(2 * P, D_MODEL)).astype(mmdt),
                "tri2": tri2.astype(mmdt),
            }
        )
    return in_maps


def kernel(residual, q, k, v, W_O, b_O, _trace=False, _trace_kwargs=None):
    global LAST_RESULTS
    residual = np.asarray(residual, dtype=np.float32)
    B = residual.shape[0]
    in_maps = make_in_maps(q, k, v, W_O)
    nc = _get_program()
    res = run_bass_kernel_spmd(
        nc, in_maps, list(range(N_CORES)), trace=_trace, **(_trace_kwargs or {})
    )
    LAST_RESULTS = res
    out = np.zeros((B, S, D_MODEL), dtype=np.float32)
    for core in range(N_CORES):
        out[core // 4] += res.results[core]["pout"].astype(np.float32)
    out += np.asarray(b_O, dtype=np.float32)
    return (residual, out.astype(np.float32))
